# revision 27
# baseline (speedup 1.0000x reference)
"""CRLoss (hard-negative triplet mining over a [B,B] similarity matrix) on 8 trn2 cores.

Sharding: rows of `similarity` split across 8 cores (1024 rows each). Labels
replicated. Similarity is converted to fp16 host-side: the mined hardest-
negative values then carry at most one fp16 ulp (~1e-3) of error each, which
largely cancels across 16K rows (total rel err ~1e-6); the anchor-positive
diagonal and all loss arithmetic stay exact f32 on host.

Per core all 8 row-tiles of [128, 8192] fp16 live in SBUF at once (16 MB +
labels + scratch < 24 MB), loaded by 4 chunk DMAs. No SBUF location is written
by more than one DMA, and every compute buffer has a single writer per tile
step on a single engine (DVE). This matters because this compiler build
encodes only ONE sync-wait per instruction: slot reuse or multi-engine
consumers would need two. "Absorber" copies observe each chunk-DMA semaphore
on DVE before the chunk's first real consumer.

Compute per tile (DVE):
  - scalar_tensor_tensor: masked = (label[col] != label[row]) * sim
    (multiply by 1.0/0.0 - exact)
  - tensor_reduce(max) -> per-row hardest negative "an"
  - tensor_max running column max -> per-core column partials
Host: combine per-core column partials, then the O(B) loss math in f32.
"""

import os

import numpy as np

B = 8192
N_CORES = 8
ROWS_PER_CORE = B // N_CORES  # 1024
P = 128
N_TILES = ROWS_PER_CORE // P  # 8
NCH = 4  # sim loaded in NCH chunk DMAs of N_TILES/NCH tiles each
# "i16": fixed-point int16 (SCALE quantization, ~1e-7 total rel err)
# "f16": float16 (faster if 16-bit float DVE perf modes engage; ~1e-5 err)
DTYPE = os.environ.get("CRL_DTYPE", "i16")
SCALE = 5000.0 if DTYPE == "i16" else 1.0

_cache: dict = {}
last_results = None  # BassKernelResults from the most recent run (for test.py)


def _build_bass():
    import concourse.bass as bass
    import concourse.mybir as mybir
    from concourse.tile import TileContext

    i16 = mybir.dt.int16 if DTYPE == "i16" else mybir.dt.float16
    nc = bass.Bass(target_bir_lowering=False)

    sim = nc.dram_tensor("sim", [N_TILES, P, B], i16, kind="ExternalInput")
    # columns 0..B-1: per-column labels (same in every partition);
    # column B+t: labels of tile t's 128 rows. Values in [-2048, 2047] (exact).
    labs = nc.dram_tensor("labs", [P, B + N_TILES], i16, kind="ExternalInput")
    # one extra (garbage) column on each output: the out-DMA reads it, and a
    # DVE memset of it afterwards observes the out-DMA's semaphore (WAR) so
    # the kernel-tail drain can wait on the DVE semaphore alone.
    row_an = nc.dram_tensor("row_an", [P, N_TILES + 1], i16, kind="ExternalOutput")
    colmax = nc.dram_tensor("colmax", [P, B + 1], i16, kind="ExternalOutput")

    tpc = N_TILES // NCH

    with TileContext(nc) as tc:
        with tc.tile_pool(name="pp", bufs=1) as pp:
            lt = pp.tile([P, B + N_TILES], i16, tag="labs")
            an_t = pp.tile([P, N_TILES + 1], i16, tag="an")
            sa = pp.tile([P, N_TILES * B], i16, tag="simall")
            mk = pp.tile([P, B], i16, tag="mk")  # masked tile (DVE-private)
            acc = pp.tile([P, B + 1], i16, tag="acc")  # running col max
            absorb = pp.tile([P, 1], i16, tag="absorb")

            nc.sync.dma_start(out=lt[:], in_=labs[:])
            # Observe the labs-DMA semaphore on DVE before any real consumer.
            nc.vector.tensor_copy(absorb[:], lt[:, :1])

            for k in range(NCH):
                nc.sync.dma_start(
                    out=sa[:, k * tpc * B : (k + 1) * tpc * B].rearrange(
                        "p (t j) -> p t j", j=B
                    ),
                    in_=sim[k * tpc : (k + 1) * tpc].rearrange("t p j -> p t j"),
                )

            for t in range(N_TILES):
                raw = sa[:, t * B : (t + 1) * B]
                if t % tpc == 0:
                    # Observe this chunk's DMA semaphore on DVE: single-cell
                    # self-copy inside the chunk. Its only dependency is the
                    # chunk DMA; the chunk's consumers are ordered after it
                    # through the engine's own semaphore, which they already
                    # carry.
                    nc.vector.tensor_copy(raw[:, :1], raw[:, :1])
                # masked = (labcol != labrow) * sim
                nc.vector.scalar_tensor_tensor(
                    out=mk[:],
                    in0=lt[:, :B],
                    scalar=lt[:, B + t : B + t + 1],
                    in1=raw,
                    op0=mybir.AluOpType.not_equal,
                    op1=mybir.AluOpType.mult,
                )
                nc.vector.tensor_reduce(
                    an_t[:, t : t + 1],
                    mk[:],
                    mybir.AxisListType.X,
                    mybir.AluOpType.max,
                )
                if t == 0:
                    nc.vector.tensor_copy(acc[:, :B], mk[:])
                else:
                    nc.vector.tensor_max(acc[:, :B], acc[:, :B], mk[:])

            # Output DMAs on the Activation HWDGE queue (fresh semaphores).
            nc.scalar.dma_start(out=row_an[:], in_=an_t[:])
            nc.scalar.dma_start(out=colmax[:], in_=acc[:])
            # Observe each out-DMA's semaphore on DVE by overwriting the
            # garbage column it read (pure WAR dependency: one wait each).
            nc.vector.memset(an_t[:, N_TILES:], 0)
            nc.vector.memset(acc[:, B:], 0)

    _fix_tail_drain(nc)
    return nc


def _fix_tail_drain(nc):
    """This walrus build encodes a single sync-wait per instruction, but the
    kernel-tail drain waits on every DMA semaphore plus the DVE semaphore.
    Every DMA semaphore is observed by a DVE instruction (absorber copies for
    loads, garbage-column memsets for stores), so the DVE-semaphore wait alone
    transitively implies all of them: drop the rest."""
    dma_sems = set()
    for ins in nc.inst_map.values():
        if type(ins).__name__ == "InstDMACopy":
            si = getattr(ins, "sync_info", None)
            for u in (getattr(si, "on_update", None) or []):
                dma_sems.add(u.id)
    for ins in nc.inst_map.values():
        if type(ins).__name__ == "InstDrain":
            si = getattr(ins, "sync_info", None)
            w = (getattr(si, "on_wait", None) or []) if si else []
            if len(w) > 1:
                keep = [x for x in w if x.id not in dma_sems]
                assert len(keep) == 1, [(x.id, x.wait_value) for x in w]
                si.on_wait = keep


def kernel(similarity, labels, margin, semi):
    global last_results
    from concourse.bass_utils import run_bass_kernel_spmd

    sim = np.asarray(similarity, dtype=np.float32)
    lab = np.asarray(labels).reshape(-1)
    marg = np.asarray(margin, dtype=np.float32).reshape(-1)

    # Dense-rank labels into [-2048, 2047] (exact in both int16 and fp16;
    # equality preserved).
    np_dt = np.int16 if DTYPE == "i16" else np.float16
    _, inv = np.unique(lab, return_inverse=True)
    lab16 = (inv.astype(np.int32) - 2048).astype(np_dt)
    labcols = np.broadcast_to(lab16[None, :], (P, B))

    # Fixed-point int16 encoding of the similarity matrix. Masking multiplies
    # by 0/1 and max-mining is order-preserving, so the mined values carry
    # only the +-1e-4 quantization of this rounding - no fp16 max-selection
    # bias. Host arithmetic stays f32 and the diagonal is exact.
    if DTYPE == "i16":
        sim16 = np.clip(np.rint(sim * SCALE), -32700, 32700).astype(np.int16)
    else:
        sim16 = sim.astype(np.float16)

    if "nc" not in _cache:
        _cache["nc"] = _build_bass()
    nc = _cache["nc"]

    in_maps = []
    for c in range(N_CORES):
        r0 = c * ROWS_PER_CORE
        shard = sim16[r0 : r0 + ROWS_PER_CORE].reshape(N_TILES, P, B)
        lr = lab16[r0 : r0 + ROWS_PER_CORE].reshape(N_TILES, P).T  # [P, N_TILES]
        labs = np.ascontiguousarray(
            np.concatenate([labcols, lr], axis=1, dtype=np_dt)
        )
        in_maps.append({"sim": shard, "labs": labs})

    trace = os.environ.get("CRL_TRACE", "0") == "1"
    res = run_bass_kernel_spmd(
        nc, in_maps, core_ids=list(range(N_CORES)), trace=trace
    )
    last_results = res

    # an for row r = c*1024 + t*128 + p  at row_an[p, t]; drop garbage column
    inv_s = np.float32(1.0 / SCALE)
    an_row = np.concatenate(
        [r["row_an"][:, :N_TILES].astype(np.float32).T.reshape(-1) for r in res.results]
    ) * inv_s  # [B]
    colp = np.stack([r["colmax"][:, :B] for r in res.results]).astype(np.float32)
    an_col = colp.reshape(N_CORES * P, B).max(axis=0) * inv_s  # [B]

    ap = np.ascontiguousarray(np.diagonal(sim))
    mam = marg - ap  # f32

    def one_side(an):
        valid = an > ap
        loss = np.maximum(mam + an, np.float32(0.0))
        return np.where(valid, loss, np.float32(0.0)).sum(dtype=np.float32)

    total = np.float32(one_side(an_row)) + np.float32(one_side(an_col))
    return np.asarray(total, dtype=np.float32)


# revision 28
# speedup vs baseline: 1.0109x; 1.0109x over previous
"""CRLoss (hard-negative triplet mining over a [B,B] similarity matrix) on 8 trn2 cores.

Sharding: rows of `similarity` split across 8 cores (1024 rows each). Labels
replicated. Similarity is converted to fp16 host-side: the mined hardest-
negative values then carry at most one fp16 ulp (~1e-3) of error each, which
largely cancels across 16K rows (total rel err ~1e-6); the anchor-positive
diagonal and all loss arithmetic stay exact f32 on host.

Per core all 8 row-tiles of [128, 8192] fp16 live in SBUF at once (16 MB +
labels + scratch < 24 MB), loaded by 4 chunk DMAs. No SBUF location is written
by more than one DMA, and every compute buffer has a single writer per tile
step on a single engine (DVE). This matters because this compiler build
encodes only ONE sync-wait per instruction: slot reuse or multi-engine
consumers would need two. "Absorber" copies observe each chunk-DMA semaphore
on DVE before the chunk's first real consumer.

Compute per tile (DVE):
  - scalar_tensor_tensor: masked = (label[col] != label[row]) * sim
    (multiply by 1.0/0.0 - exact)
  - tensor_reduce(max) -> per-row hardest negative "an"
  - tensor_max running column max -> per-core column partials
Host: combine per-core column partials, then the O(B) loss math in f32.
"""

import os

import numpy as np

B = 8192
N_CORES = 8
ROWS_PER_CORE = B // N_CORES  # 1024
P = 128
N_TILES = ROWS_PER_CORE // P  # 8
NCH = 4  # sim loaded in NCH chunk DMAs of N_TILES/NCH tiles each
# "i16": fixed-point int16 (SCALE quantization, ~1e-7 total rel err)
# "f16": float16 (faster if 16-bit float DVE perf modes engage; ~1e-5 err)
DTYPE = os.environ.get("CRL_DTYPE", "i16")
SCALE = 5000.0 if DTYPE == "i16" else 1.0

_cache: dict = {}
last_results = None  # BassKernelResults from the most recent run (for test.py)


def _build_bass():
    import concourse.bass as bass
    import concourse.mybir as mybir
    from concourse.tile import TileContext

    i16 = mybir.dt.int16 if DTYPE == "i16" else mybir.dt.float16
    nc = bass.Bass(target_bir_lowering=False)

    sim = nc.dram_tensor("sim", [N_TILES, P, B], i16, kind="ExternalInput")
    # columns 0..B-1: per-column labels (same in every partition);
    # column B+t: labels of tile t's 128 rows. Values in [-2048, 2047] (exact).
    labs = nc.dram_tensor("labs", [P, B + N_TILES], i16, kind="ExternalInput")
    # one extra (garbage) column on each output: the out-DMA reads it, and a
    # DVE memset of it afterwards observes the out-DMA's semaphore (WAR) so
    # the kernel-tail drain can wait on the DVE semaphore alone.
    row_an = nc.dram_tensor("row_an", [P, N_TILES + 1], i16, kind="ExternalOutput")
    colmax = nc.dram_tensor("colmax", [P, B + 1], i16, kind="ExternalOutput")

    tpc = N_TILES // NCH

    with TileContext(nc) as tc:
        with tc.tile_pool(name="pp", bufs=1) as pp:
            lt = pp.tile([P, B + N_TILES], i16, tag="labs")
            an_t = pp.tile([P, N_TILES + 1], i16, tag="an")
            sa = pp.tile([P, N_TILES * B], i16, tag="simall")
            mk = pp.tile([P, B], i16, tag="mk")  # masked tile (DVE-private)
            acc = pp.tile([P, B + 1], i16, tag="acc")  # running col max
            absorb = pp.tile([P, 1], i16, tag="absorb")

            nc.sync.dma_start(out=lt[:], in_=labs[:])
            # Observe the labs-DMA semaphore on DVE before any real consumer.
            nc.vector.tensor_copy(absorb[:], lt[:, :1])

            for k in range(NCH):
                nc.sync.dma_start(
                    out=sa[:, k * tpc * B : (k + 1) * tpc * B].rearrange(
                        "p (t j) -> p t j", j=B
                    ),
                    in_=sim[k * tpc : (k + 1) * tpc].rearrange("t p j -> p t j"),
                )

            for t in range(N_TILES):
                raw = sa[:, t * B : (t + 1) * B]
                md = acc[:, :B] if t == 0 else mk[:]
                if t % tpc == 0:
                    # Observe this chunk's DMA semaphore on DVE: single-cell
                    # self-copy inside the chunk. Its only dependency is the
                    # chunk DMA; the chunk's consumers are ordered after it
                    # through the engine's own semaphore, which they already
                    # carry.
                    nc.vector.tensor_copy(raw[:, :1], raw[:, :1])
                # masked = (labcol != labrow) * sim
                # tile 0 writes the masked values straight into the
                # accumulator: saves one full copy pass
                nc.vector.scalar_tensor_tensor(
                    out=md,
                    in0=lt[:, :B],
                    scalar=lt[:, B + t : B + t + 1],
                    in1=raw,
                    op0=mybir.AluOpType.not_equal,
                    op1=mybir.AluOpType.mult,
                )
                nc.vector.tensor_reduce(
                    an_t[:, t : t + 1],
                    md,
                    mybir.AxisListType.X,
                    mybir.AluOpType.max,
                )
                if t > 0:
                    nc.vector.tensor_max(acc[:, :B], acc[:, :B], mk[:])

            # Output DMAs on the Activation HWDGE queue (fresh semaphores).
            nc.scalar.dma_start(out=row_an[:], in_=an_t[:])
            nc.scalar.dma_start(out=colmax[:], in_=acc[:])
            # Observe each out-DMA's semaphore on DVE by overwriting the
            # garbage column it read (pure WAR dependency: one wait each).
            nc.vector.memset(an_t[:, N_TILES:], 0)
            nc.vector.memset(acc[:, B:], 0)

    _fix_tail_drain(nc)
    return nc


def _fix_tail_drain(nc):
    """This walrus build encodes a single sync-wait per instruction, but the
    kernel-tail drain waits on every DMA semaphore plus the DVE semaphore.
    Every DMA semaphore is observed by a DVE instruction (absorber copies for
    loads, garbage-column memsets for stores), so the DVE-semaphore wait alone
    transitively implies all of them: drop the rest."""
    dma_sems = set()
    for ins in nc.inst_map.values():
        if type(ins).__name__ == "InstDMACopy":
            si = getattr(ins, "sync_info", None)
            for u in (getattr(si, "on_update", None) or []):
                dma_sems.add(u.id)
    for ins in nc.inst_map.values():
        if type(ins).__name__ == "InstDrain":
            si = getattr(ins, "sync_info", None)
            w = (getattr(si, "on_wait", None) or []) if si else []
            if len(w) > 1:
                keep = [x for x in w if x.id not in dma_sems]
                assert len(keep) == 1, [(x.id, x.wait_value) for x in w]
                si.on_wait = keep


def kernel(similarity, labels, margin, semi):
    global last_results
    from concourse.bass_utils import run_bass_kernel_spmd

    sim = np.asarray(similarity, dtype=np.float32)
    lab = np.asarray(labels).reshape(-1)
    marg = np.asarray(margin, dtype=np.float32).reshape(-1)

    # Dense-rank labels into [-2048, 2047] (exact in both int16 and fp16;
    # equality preserved).
    np_dt = np.int16 if DTYPE == "i16" else np.float16
    _, inv = np.unique(lab, return_inverse=True)
    lab16 = (inv.astype(np.int32) - 2048).astype(np_dt)
    labcols = np.broadcast_to(lab16[None, :], (P, B))

    # Fixed-point int16 encoding of the similarity matrix. Masking multiplies
    # by 0/1 and max-mining is order-preserving, so the mined values carry
    # only the +-1e-4 quantization of this rounding - no fp16 max-selection
    # bias. Host arithmetic stays f32 and the diagonal is exact.
    if DTYPE == "i16":
        sim16 = np.clip(np.rint(sim * SCALE), -32700, 32700).astype(np.int16)
    else:
        sim16 = sim.astype(np.float16)

    if "nc" not in _cache:
        _cache["nc"] = _build_bass()
    nc = _cache["nc"]

    in_maps = []
    for c in range(N_CORES):
        r0 = c * ROWS_PER_CORE
        shard = sim16[r0 : r0 + ROWS_PER_CORE].reshape(N_TILES, P, B)
        lr = lab16[r0 : r0 + ROWS_PER_CORE].reshape(N_TILES, P).T  # [P, N_TILES]
        labs = np.ascontiguousarray(
            np.concatenate([labcols, lr], axis=1, dtype=np_dt)
        )
        in_maps.append({"sim": shard, "labs": labs})

    trace = os.environ.get("CRL_TRACE", "0") == "1"
    res = run_bass_kernel_spmd(
        nc, in_maps, core_ids=list(range(N_CORES)), trace=trace
    )
    last_results = res

    # an for row r = c*1024 + t*128 + p  at row_an[p, t]; drop garbage column
    inv_s = np.float32(1.0 / SCALE)
    an_row = np.concatenate(
        [r["row_an"][:, :N_TILES].astype(np.float32).T.reshape(-1) for r in res.results]
    ) * inv_s  # [B]
    colp = np.stack([r["colmax"][:, :B] for r in res.results]).astype(np.float32)
    an_col = colp.reshape(N_CORES * P, B).max(axis=0) * inv_s  # [B]

    ap = np.ascontiguousarray(np.diagonal(sim))
    mam = marg - ap  # f32

    def one_side(an):
        valid = an > ap
        loss = np.maximum(mam + an, np.float32(0.0))
        return np.where(valid, loss, np.float32(0.0)).sum(dtype=np.float32)

    total = np.float32(one_side(an_row)) + np.float32(one_side(an_col))
    return np.asarray(total, dtype=np.float32)
